# revision 29
# baseline (speedup 1.0000x reference)
"""Multi-head linear attention (Performer/FAVOR+) Bass kernel for 8x TRN2 cores.

Sharding: 8 cores = 4 batches x 2 head-groups. Core c handles batch c//2 and
heads [4*(c%2), 4*(c%2)+4).

Math notes (exact rewrites of the reference, not approximations):
  - omega is sqrt(64) * orthogonal, so 0.5*||q||^2 = ||q @ Omega.T||^2 / 128:
    the squared-sum term is computed from xw itself.
  - The per-row scale exp(-qsq_t), the global 1/sqrt(128) scale and
    (approximately) the +EPS term all cancel in out = qkv[..,:64]/qkv[..,64],
    so the q-side feature map is just exp(+-xw).
  - The k-side scale rho_s = exp(-ksq_s) is folded into v1 = [v, 1]*rho so
    kp is also just exp(+-kxw).

v2 structure (heads processed in PAIRS of two):
  - Q proj packs a head pair into one 128-col stationary [wqo_he | wqo_ho]:
    one N=512 matmul per (pair, chunk, tchunk) - half the PE work of the
    [w,-w] trick. The +- split becomes two ACT exps (same element count).
  - kv accumulation is TRANSPOSED: stationary = v1 (65 cols -> cheap
    LDWEIGHTS), moving = kp (N=128): kvT[d(65), h, sign, m(64)] accumulates
    in one PSUM bank over all 32 s-tiles. At the end, 8 tiny matmuls against
    an identity build block-diagonal KVP/KVN [128m, 130d] per pair
    (rows 0:64 = even head's features, 64:128 = odd head's; bank-clear on the
    first matmul zeroes the off-diagonal blocks).
  - qkv per (t-tile, pair): qp_pos.T @ KVP + qp_neg.T @ KVN -> [128t, 130]
    = both heads' 65 cols (64 values + normalizer).
  - Normalization happens ON HOST: the kernel DMAs bf16 rows
    [t, (pair, head, 65)] and the host divides values by the normalizer.
  - ksq: ACT Square into PSUM + DVE reduce (no engine can read a PSUM
    operand twice, so x*x on DVE/GPSIMD is not lowerable).
  - Input DMA: few large transfers in consumption order (k/v/q segments
    interleaved) instead of ~100 small ones.

v3 scheduling (97.8us -> ~86us; ACT op cost is (N+352)/1.2 ns; warm PE
streams 1 col/cycle at 2.4 GHz, but HAM halves the clock after ~3.4us
of idle):
  - No front q-work: the PE queue is in-order, so q chunks would
    head-of-line block the kv loop behind late qt DMA. All 8 q chunks
    run in the tail; ~28 warmup dummies bridge the preamble to the
    first kt/vt arrival (14-18us) so the kv loop starts on a warm clock.
  - kvT matmuls lag 5 iterations (kp bufs=7, v1 bufs=5) so the
    multi-engine v1 ramp (exp -> square -> reduce -> rho -> v1) never
    stalls the PE at pipeline fill; ACT emits the v1 chain (rho, sq)
    BEFORE the kp exps each iteration for the same reason.
  - Tail: DVE copies qx PSUM->SBUF immediately, so the next chunk's
    projection WAR waits on the fast copy instead of ACT's exps; exps
    read the SBUF copy; qkv lags 3 chunks; output blocks alternate
    between both HWDGE queues, with the final block split across both.
"""

import sys

import numpy as np

for _p in ("/opt/trn_rl_repo", "/root/.axon_site/_ro/trn_rl_repo"):
    try:
        import concourse  # noqa: F401
        break
    except ImportError:
        if _p not in sys.path:
            sys.path.insert(0, _p)

B, T, D, H = 4, 4096, 512, 8
DK = DV = 64
HPC = 4            # heads per core
NPAIR = 2          # head pairs per core
NCH = 4            # f chunks (512 / 128)
P = 128
ST = T // P        # 32 s-tiles
TC = 8             # t chunks
TCW = T // TC      # 512
DVN = DV + 1       # 65: values + normalizer

# Input transfer blocks (host pre-segments the DRAM layout to match):
# First k/v segment is small so the kv loop's first iteration can start
# as early as possible after the framework preamble.
KVSEG = [(0, 256), (256, 512), (512, 1024), (1024, 1536), (1536, 2560),
         (2560, 3584), (3584, T)]
QSEG = [(0, 512), (512, 1024), (1024, 2048), (2048, 3072), (3072, T)]

_CACHE = {}


def _build_program(reps=1):
    import concourse.mybir as mybir
    import concourse.tile as tile
    from concourse import bacc
    from contextlib import ExitStack

    dt = mybir.dt

    nc = bacc.Bacc("TRN2", target_bir_lowering=False, debug=False)

    # All inputs are PRE-SEGMENTED on the host into the exact transfer
    # blocks the kernel issues, each contiguous per partition: every DMA is
    # then 128 descriptors (one per partition) with a single completion
    # event, whatever its size. (Early completions serialize at ~1.5-2 us
    # under the 8-core startup HBM load, so completion COUNT on the
    # critical path is what matters.)
    qt_d = nc.dram_tensor("qt", [P, NCH * T], dt.float16, kind="ExternalInput")
    kt_d = nc.dram_tensor("kt", [P, NCH * T], dt.float16, kind="ExternalInput")
    vt_d = nc.dram_tensor("vt", [P, NCH * T], dt.float16, kind="ExternalInput")
    wqp_d = nc.dram_tensor("wqp", [P, NPAIR * NCH * P], dt.float16,
                           kind="ExternalInput")
    wko_d = nc.dram_tensor("wko", [P, NCH * HPC * DK], dt.float16,
                           kind="ExternalInput")
    wv_d = nc.dram_tensor("wv", [P, NCH * HPC * DV], dt.float16,
                          kind="ExternalInput")
    id_d = nc.dram_tensor("ident", [DVN, DVN], dt.bfloat16,
                          kind="ExternalInput")
    out_d = nc.dram_tensor("out", [T * HPC * DVN], dt.bfloat16,
                           kind="ExternalOutput")

    with tile.TileContext(nc) as tc, ExitStack() as ctx:
        const = ctx.enter_context(tc.tile_pool(name="const", bufs=1))
        work = ctx.enter_context(tc.tile_pool(name="work", bufs=3))
        psum = ctx.enter_context(tc.tile_pool(name="psum", bufs=1, space="PSUM"))
        for _rep in range(reps):
            _emit_body(nc, tc, const, work, psum, mybir, dt,
                       qt_d, kt_d, vt_d, wqp_d, wko_d, wv_d, id_d, out_d)

    nc.compile()
    return nc


def _emit_body(nc, tc, const, work, psum, mybir, dt,
               qt_d, kt_d, vt_d, wqp_d, wko_d, wv_d, id_d, out_d):
    AF = mybir.ActivationFunctionType

    # persistent SBUF residents
    qt = const.tile([P, NCH, T], dt.float16)
    kt = const.tile([P, NCH, T], dt.float16)
    vt = const.tile([P, NCH, T], dt.float16)
    wqp = const.tile([P, NPAIR, NCH, P], dt.float16)
    wko = const.tile([P, NCH, HPC * DK], dt.float16)
    wv = const.tile([P, NCH, HPC * DV], dt.float16)
    ident = const.tile([DVN, DVN], dt.bfloat16)
    kvt_sb = const.tile([DVN, HPC, 2, DK], dt.bfloat16)
    kvp_sb = const.tile([P, NPAIR, 2 * DVN], dt.bfloat16)
    kvn_sb = const.tile([P, NPAIR, 2 * DVN], dt.bfloat16)

    # Input DMA: two HWDGE queues, each issued in consumption order.
    # HWDGE issue cost is ~650ns per 128-descriptor transfer regardless
    # of size, so multi-hundred-col groups keep the issue rate above the
    # ~358 GB/s wire. qt (tail-only) queues behind all of kt/vt so it
    # cannot steal HBM bandwidth from the kv loop.
    # k/v weights go on the (otherwise idle) scalar HWDGE queue so the
    # sync queue's first transfer is already kt segment 0: both queues
    # issue in parallel right after the preamble, putting the first
    # k-projection's operands (kt0 + wko) on SBUF as early as possible.
    nc.scalar.dma_start(out=wko[:], in_=wko_d[:, :])
    nc.scalar.dma_start(out=wv[:], in_=wv_d[:, :])
    nc.scalar.dma_start(out=wqp[:], in_=wqp_d[:, :])
    off = 0
    for s, e in KVSEG:
        w = (e - s) * NCH
        nc.sync.dma_start(out=kt[:, :, s:e], in_=kt_d[:, off:off + w])
        nc.sync.dma_start(out=vt[:, :, s:e], in_=vt_d[:, off:off + w])
        off += w
    qoff = [0]
    for s, e in QSEG:
        qoff.append(qoff[-1] + (e - s) * NCH)
    for i in range(len(QSEG)):
        s, e = QSEG[i]
        nc.sync.dma_start(out=qt[:, :, s:e], in_=qt_d[:, qoff[i]:qoff[i + 1]])
    nc.gpsimd.dma_start(out=ident[:], in_=id_d[:, :])

    # PE warm-up: dummy matmuls on scratch fill the PE from the end of the
    # framework preamble (~6 us) until kt/vt segment 0 lands (~11.5 us).
    # HAM un-throttles the PE clock (1.2 -> 2.4 GHz) after ~3.4 us of
    # sustained activity, so the kv loop starts warm. The dummies write
    # the kvt bank, which the memset below re-zeroes. No front q-work:
    # qt streams AFTER kt/vt, and in-order PE queues mean any q-chunk
    # emitted here would head-of-line block the kv loop on late qt data.
    kvt_ps = psum.tile([DVN, HPC, 2, DK], dt.float32, tag="kvt", bufs=1)
    scr = const.tile([P, TCW], dt.bfloat16)
    nc.gpsimd.memset(scr[:], 0.0)
    kvt_flat = kvt_ps[:].rearrange("p h s m -> p (h s m)")
    # 32 dummies bridge the PE from the end of the preamble (~3 us) to
    # just before the first kt/vt arrival (14-18 us, run-variable): ~8
    # run cold (3.4 us warms HAM), the rest at 2.4 GHz; any remaining
    # idle stays within HAM's 3.4 us window, so the kv loop starts warm.
    for _w in range(32):
        nc.tensor.matmul(kvt_flat[0:DVN, :], scr[:, 0:DVN], scr[:],
                         start=True, stop=True, skip_group_check=True)

    qp_tiles = {}

    def emit_q_chunk(tcx):
        tsl = slice(tcx * TCW, (tcx + 1) * TCW)
        qx = psum.tile([P, NPAIR, TCW], dt.float32, tag="kxwv", bufs=3,
                       name="qx")
        for pr in range(NPAIR):
            for c in range(NCH):
                nc.tensor.matmul(
                    qx[:, pr, :], wqp[:, pr, c, :], qt[:, c, tsl],
                    start=(c == 0), stop=(c == NCH - 1),
                    skip_group_check=True,
                )
        # DVE copies qx out of PSUM immediately: the qx slot is then freed
        # by the (fast) copy instead of by ACT's exps, so the next chunk's
        # projection never waits on ACT (PSUM WAR decoupling). fp16 halves
        # the DVE cost; the resulting xw quantization noise cancels in
        # out = qkv[:,:64]/qkv[:,64] (numerator and denominator share the
        # dominant qp feature weights), so only k-side xw must stay fp32.
        qxs = work.tile([P, NPAIR, TCW], dt.float16, tag="qxs", bufs=2,
                        name=f"qxs{tcx}")
        nc.vector.tensor_copy(qxs[:], qx[:])
        qpp = work.tile([P, NPAIR, TCW], dt.bfloat16, tag="qpp", bufs=4,
                        name=f"qpp{tcx}")
        qpn = work.tile([P, NPAIR, TCW], dt.bfloat16, tag="qpn", bufs=4,
                        name=f"qpn{tcx}")
        # ACT op cost is (N + 352)/1.2 ns, so batch the whole chunk into
        # one op per sign. Only the LAST chunk is split per half so the
        # final qkv matmuls can start after half the exps (shorter drain).
        if tcx == TC - 1:
            hw = TCW // 2
            for th in range(2):
                hsl = slice(th * hw, (th + 1) * hw)
                nc.scalar.activation(qpp[:, :, hsl], qxs[:, :, hsl], AF.Exp)
                nc.scalar.activation(qpn[:, :, hsl], qxs[:, :, hsl], AF.Exp,
                                     scale=-1.0)
        else:
            nc.scalar.activation(qpp[:], qxs[:], AF.Exp)
            nc.scalar.activation(qpn[:], qxs[:], AF.Exp, scale=-1.0)
        qp_tiles[tcx] = (qpp, qpn)

    # kvT accumulator: one PSUM bank, [d(65), h, sign, m(64)] fp32.
    # All matmuls use start=False and accumulate; memset clears the bank
    # (overwrite-on-clear / add-on-set per-element semantics).
    nc.vector.memset(kvt_ps[:], 0.0)

    # ---------------- phase KV (with q-projection work interleaved) -----
    # Two s-tiles per "pair" iteration. Engines execute queues IN ORDER;
    # cross-engine deps are software-pipelined: v1 (needs rho from ACT) is
    # emitted one iteration late on DVE, the kvT matmuls (need v1) one
    # iteration later still on PE.
    NP_ = ST // 2    # 16 iterations
    stage = {}

    def emit_rho(pi):
        st_ = stage[pi]
        rho = work.tile([P, 2, HPC, 1], dt.float32, tag="rho", bufs=3)
        nc.scalar.activation(rho[:], st_["ksqr"][:], AF.Exp,
                             scale=-1.0 / 128.0)
        st_["rho"] = rho

    def emit_v1(pi):
        st_ = stage[pi]
        v1 = work.tile([P, 2, HPC, DVN], dt.bfloat16, tag="v1", name="v1",
                       bufs=5)
        nc.vector.tensor_mul(
            v1[:, :, :, 0:DV], st_["v_ps"],
            st_["rho"][:].broadcast_to([P, 2, HPC, DV])
        )
        nc.vector.tensor_copy(v1[:, :, :, DV:DVN], st_["rho"][:])
        st_["v1"] = v1

    def emit_kvt(pi):
        st_ = stage.pop(pi)
        for p_ in range(2):
            si = 2 * pi + p_
            for h in range(HPC):
                # out = v1_h.T @ kp_h : [65, (sign,64)]
                nc.tensor.matmul(
                    kvt_ps[:, h, :, :], st_["v1"][:, p_, h, :],
                    st_["kp"][:, p_, :, h * DK:(h + 1) * DK],
                    start=False, stop=(si == ST - 1),
                    skip_group_check=True,
                )

    for pi in range(NP_):
        # Software pipeline (per-engine FIFOs never head-of-line block):
        #   iteration pi runs   k/v-proj(pi) | kp(pi) ACT | square(pi)
        #   GPSIMD | reduce(pi) DVE | rho(pi-1) ACT | v1(pi-2) DVE |
        #   kvT(pi-5) PE.
        # v1(pi-2) is emitted FIRST so the DVE completes it before the
        # k-proj matmuls of pi need its kxwv slot (WAR, bufs=3).
        # The kvT matmuls lag 5 iterations so the multi-engine ramp of
        # the v1 chain (exp -> square -> reduce -> rho -> v1) never
        # stalls the PE at pipeline fill.
        if pi >= 2:
            emit_v1(pi - 2)
        if pi >= 5:
            emit_kvt(pi - 5)

        # kxw and v share PSUM banks: [..., 0:64] = kxw, 64:128 = v.
        # bufs=3: v1 lags two pairs, so the slot of pair pi-3 must stay
        # readable until v1(pi-3) has run.
        kxwv = psum.tile([P, 2, HPC, 2 * DK], dt.float32, tag="kxwv",
                         bufs=3, name="kxwv")
        kxw = kxwv[:, :, :, 0:DK]
        v_ps = kxwv[:, :, :, DK:2 * DK]
        for p_ in range(2):
            ssl = slice((2 * pi + p_) * P, (2 * pi + p_ + 1) * P)
            for c in range(NCH):
                nc.tensor.matmul(
                    kxwv[:, p_, :, 0:DK], kt[:, c, ssl], wko[:, c, :],
                    start=(c == 0), stop=(c == NCH - 1),
                )
            for c in range(NCH):
                nc.tensor.matmul(
                    kxwv[:, p_, :, DK:2 * DK], vt[:, c, ssl], wv[:, c, :],
                    start=(c == 0), stop=(c == NCH - 1),
                )

        # ACT queue order per iteration: rho(pi-1), sq(pi), kp+-(pi).
        # The v1 chain (sq -> reduce -> rho -> v1) paces the kxwv slot
        # rotation (PE WAR), so it gets ACT priority; the kp exps only
        # feed the kvT matmuls, which lag 5 iterations.
        if pi >= 1:
            emit_rho(pi - 1)
        sqsc = psum.tile([P, 2, HPC, DK], dt.float32, tag="sq", bufs=1,
                         name="sqsc")
        nc.scalar.activation(sqsc[:], kxw, AF.Square)
        ksqr = work.tile([P, 2, HPC, 1], dt.float32, tag="ksqr", bufs=3)
        nc.vector.reduce_sum(ksqr[:], sqsc[:], axis=mybir.AxisListType.X)

        # kp: [s, p_, sign, m(4h x 64)]
        kp = work.tile([P, 2, 2, HPC * DK], dt.bfloat16, tag="kp", bufs=7)
        nc.scalar.activation(kp[:, :, 0, :], kxw, AF.Exp, scale=1.0)
        nc.scalar.activation(kp[:, :, 1, :], kxw, AF.Exp, scale=-1.0)
        stage[pi] = {"v_ps": v_ps, "ksqr": ksqr, "kp": kp}

    emit_rho(NP_ - 1)
    emit_v1(NP_ - 2)
    # q-chunk 0/1 projections fill the PE while the DVE/ACT drain chain
    # (rho/v1 of the last pairs) completes.
    emit_q_chunk(0)
    emit_v1(NP_ - 1)
    emit_kvt(NP_ - 5)
    emit_kvt(NP_ - 4)
    emit_kvt(NP_ - 3)
    emit_q_chunk(1)
    emit_kvt(NP_ - 2)
    emit_kvt(NP_ - 1)

    # ---------------- kv fixup: kvT -> block-diagonal KVP / KVN ---------
    # memset zeroes the data; matmuls accumulate with start=False (correct
    # for both stale has_written states); the never-written off-diagonal
    # blocks keep the memset zeros.
    nc.vector.tensor_copy(kvt_sb[:], kvt_ps[:])
    for pr in range(NPAIR):
        he, ho = 2 * pr, 2 * pr + 1
        kvx_ps = psum.tile([P, 2 * DVN], dt.float32, tag="sq", bufs=1,
                           name=f"kvx{pr}")
        nc.vector.memset(kvx_ps[:], 0.0)
        nc.tensor.matmul(kvx_ps[0:DK, 0:DVN],
                         kvt_sb[:, he, 0, :], ident[:],
                         start=False, stop=False, skip_group_check=True)
        nc.tensor.matmul(kvx_ps[DK:P, DVN:2 * DVN],
                         kvt_sb[:, ho, 0, :], ident[:],
                         start=False, stop=True,
                         skip_group_check=True, tile_position=(0, DK))
        nc.vector.tensor_copy(kvp_sb[:, pr, :], kvx_ps[:])
        kvy_ps = psum.tile([P, 2 * DVN], dt.float32, tag="kvt", bufs=1,
                           name=f"kvy{pr}")
        nc.vector.memset(kvy_ps[:], 0.0)
        nc.tensor.matmul(kvy_ps[0:DK, 0:DVN],
                         kvt_sb[:, he, 1, :], ident[:],
                         start=False, stop=False, skip_group_check=True)
        nc.tensor.matmul(kvy_ps[DK:P, DVN:2 * DVN],
                         kvt_sb[:, ho, 1, :], ident[:],
                         start=False, stop=True,
                         skip_group_check=True, tile_position=(0, DK))
        nc.vector.tensor_copy(kvn_sb[:, pr, :], kvy_ps[:])

    # ---------------- tail: qkv + store ---------------------------------
    # (q-chunks 0-1 already ran in the front; 2-7 run here, with the qkv
    # matmuls for tchunk t emitted two tchunks late so the PE never waits
    # on the exps. Normalization is on host.)
    def emit_qkv(tcx):
        # qkv PSUM tile: [t, tti, pair, 256-pad] = 2 banks; each (tti,
        # pair) block is 130 fp32 inside its own 1 KiB half-bank, so no
        # matmul output straddles a bank. start=True on each bank's first
        # matmul clears that bank's has_written; the other blocks
        # overwrite-on-clear / add-on-set.
        qpp, qpn = qp_tiles[tcx]
        oq = work.tile([P, 4, NPAIR, 2 * DVN], dt.bfloat16, tag="oq", bufs=3)
        for th in range(2):      # two tt-halves of 2 t-tiles each
            qkv = psum.tile([P, 2, NPAIR, 2 * P], dt.float32,
                            tag="kxwv", bufs=3, name="qkv")
            for tti in range(2):
                tt = 2 * th + tti
                ttsl = slice(tt * P, (tt + 1) * P)
                for pr in range(NPAIR):
                    nc.tensor.matmul(
                        qkv[:, tti, pr, 0:2 * DVN], qpp[:, pr, ttsl],
                        kvp_sb[:, pr, :],
                        start=(pr == 0), stop=False,
                        skip_group_check=True,
                    )
                    nc.tensor.matmul(
                        qkv[:, tti, pr, 0:2 * DVN], qpn[:, pr, ttsl],
                        kvn_sb[:, pr, :],
                        start=False, stop=(pr == NPAIR - 1),
                        skip_group_check=True,
                    )
            nc.vector.tensor_copy(oq[:, 2 * th:2 * th + 2, :, :],
                                  qkv[:, :, :, 0:2 * DVN])
            # output block (tcx, th): [p, (tt, pair, 65)] contiguous per
            # partition -> 128 descriptors; host reassembles. Blocks
            # alternate between the two HWDGE queues (the scalar queue is
            # idle after the input weights) to halve the per-transfer
            # issue serialization at the drain.
            blk = tcx * 2 + th
            ofs = blk * P * 2 * HPC * DVN
            bw = 2 * HPC * DVN
            if blk == 2 * TC - 1:
                # final block: split across both queues so the last write
                # (which gates the end-of-kernel barrier) halves.
                oqv = oq[:, 2 * th:2 * th + 2, :, :].rearrange(
                    "p a b c -> p (a b c)")
                for half, eng in ((0, nc.sync), (1, nc.scalar)):
                    hsl = slice(half * (bw // 2), (half + 1) * (bw // 2))
                    eng.dma_start(
                        out=out_d.ap()[ofs:ofs + P * bw].rearrange(
                            "(p d) -> p d", p=P)[:, hsl],
                        in_=oqv[:, hsl],
                    )
            else:
                eng = nc.sync if blk % 2 == 0 else nc.scalar
                eng.dma_start(
                    out=out_d.ap()[ofs:ofs + P * bw].rearrange(
                        "(p d) -> p d", p=P
                    ),
                    in_=oq[:, 2 * th:2 * th + 2, :, :],
                )

    # lag-3 pipeline: qkv(c) is emitted after proj(c+3), so the PE never
    # waits on ACT exps; the three trailing qkvs drain while ACT finishes
    # the last chunk's exps.
    emit_q_chunk(2)
    emit_q_chunk(3)
    emit_qkv(0)
    for tcx in range(4, TC):
        emit_q_chunk(tcx)
        emit_qkv(tcx - 3)
    emit_qkv(TC - 3)
    emit_qkv(TC - 2)
    emit_qkv(TC - 1)


def _get_program(reps=1):
    if reps not in _CACHE:
        _CACHE[reps] = _build_program(reps)
    return _CACHE[reps]


def _seg_blocks(xT, segs):
    """(512, T) f-major -> [128, sum(4*w)] pre-segmented transfer layout."""
    arr = xT.reshape(NCH, P, T)                                # (c, p, t)
    blocks = [
        np.ascontiguousarray(arr[:, :, s:e].transpose(1, 0, 2)).reshape(P, -1)
        for s, e in segs
    ]
    return np.ascontiguousarray(np.concatenate(blocks, axis=1))


def _prep_core_inputs(query, value, key, wqo, wko, wv_w, core):
    b, hg = core // 2, core % 2
    hs = slice(hg * HPC, (hg + 1) * HPC)

    qT = query[b].T.astype(np.float16)                         # (512, 4096)
    kT = key[b].T.astype(np.float16)
    vT = value[b].T.astype(np.float16)

    wqo_c = wqo[hs]                                            # (4, 512, 64)
    wqp = np.stack([
        np.concatenate([wqo_c[2 * pr], wqo_c[2 * pr + 1]], axis=1)
        for pr in range(NPAIR)
    ])                                                         # (2, 512, 128)
    wqp = wqp.reshape(NPAIR, NCH, P, P).astype(np.float16)
    wqp = np.ascontiguousarray(
        wqp.transpose(2, 0, 1, 3)).reshape(P, NPAIR * NCH * P)

    wko_c = np.concatenate(list(wko[hs]), axis=1).astype(np.float16)
    wko_c = np.ascontiguousarray(
        wko_c.reshape(NCH, P, HPC * DK).transpose(1, 0, 2)).reshape(P, -1)
    wv_c = np.concatenate(list(wv_w[hs]), axis=1).astype(np.float16)
    wv_c = np.ascontiguousarray(
        wv_c.reshape(NCH, P, HPC * DV).transpose(1, 0, 2)).reshape(P, -1)

    import ml_dtypes
    ident = np.eye(DVN, dtype=ml_dtypes.bfloat16)

    return {"qt": _seg_blocks(qT, QSEG), "kt": _seg_blocks(kT, KVSEG),
            "vt": _seg_blocks(vT, KVSEG),
            "wqp": wqp, "wko": wko_c, "wv": wv_c, "ident": ident}


def kernel(query, value, key, wq, wv, wk, omega):
    from concourse.bass_utils import run_bass_kernel_spmd

    query = np.asarray(query, np.float32)
    value = np.asarray(value, np.float32)
    key = np.asarray(key, np.float32)
    wq = np.asarray(wq, np.float32)
    wv = np.asarray(wv, np.float32)
    wk = np.asarray(wk, np.float32)
    omega = np.asarray(omega, np.float32)

    nc = _get_program()

    wqo = np.einsum("hfk,mk->hfm", wq, omega)                  # (8, 512, 64)
    wko = np.einsum("hfk,mk->hfm", wk, omega)

    in_maps = [
        _prep_core_inputs(query, value, key, wqo, wko, wv, core)
        for core in range(8)
    ]
    res = run_bass_kernel_spmd(nc, in_maps, core_ids=list(range(8)))

    out = np.empty((B, T, D), np.float32)
    for core in range(8):
        b, hg = core // 2, core % 2
        qkv = np.asarray(res.results[core]["out"], np.float32)
        # blocks [tcx, th, p, tt, (h, 65)] -> t = tcx*512 + th*256 +
        # tt*128 + p
        qkv = qkv.reshape(TC, 2, P, 2, HPC, DVN).transpose(0, 1, 3, 2, 4, 5)
        qkv = qkv.reshape(T, HPC, DVN).transpose(1, 0, 2)      # (h, t, 65)
        vals = qkv[:, :, :DV] / qkv[:, :, DV:DVN]              # (4, 4096, 64)
        out[b, hg * 2048:(hg + 1) * 2048, :] = vals.reshape(2048, D)
    return out

